# revision 1
# baseline (speedup 1.0000x reference)
"""CrossAttentionBlock Trainium2 kernel — data-parallel over batch across 8 cores.

Full inputs in, full outputs out. Each core handles 2 of the 16 batch
elements; weights are replicated. No collectives.

Math notes (vs the jax reference):
- AdaRMSNorm on x: xn = x * s_x[d] * inv_rms_x[t].  inv_rms_x is a positive
  per-token scalar; q = xn @ w_q.T is later cosine-normalized per head, so
  inv_rms_x cancels (up to a negligible eps perturbation) and is skipped.
- AdaRMSNorm on crossattn_cond: the inv_rms_c factor cancels for k (cosine
  normalized) but NOT for v, so it is folded into v only.
- Cosine-sim scores are bounded (|score| <= qk_scale/sqrt(D_HEAD)), so softmax
  runs without max-subtraction; the boolean mask becomes an additive -60 bias
  inside the exp.
- Softmax denominator comes from an extra all-ones column appended to v; the
  per-(head, token) reciprocal is broadcast across partitions with a K=16
  indicator matmul.

Tokens stream through projection+attention+out-proj in 512-wide chunks
(4 chunks/core) to bound SBUF residency.
"""

import numpy as np

D_HEAD = 64
EPS = 1e-6
N, H, W, D = 16, 32, 32, 1024
L, DC, CF = 256, 1024, 768
NH = D // D_HEAD  # 16
NCORES = 8
NB = N // NCORES  # 2 batch elements per core
T = H * W  # 1024 tokens per batch element
CH = 512  # token chunk
MASK_NEG = -60.0

_cached = {}


def _build_nc():
    from contextlib import ExitStack

    import concourse.mybir as mybir
    import concourse.tile as tile
    from concourse import bacc

    f32 = mybir.dt.float32
    f32r = mybir.dt.float32r
    f16 = mybir.dt.float16
    Exp = mybir.ActivationFunctionType.Exp
    Sqrt = mybir.ActivationFunctionType.Sqrt
    MULT = mybir.AluOpType.mult
    ADD = mybir.AluOpType.add

    nc = bacc.Bacc(None, target_bir_lowering=False)

    xT = nc.declare_dram_parameter("xT", [NB, D, T], f16, isOutput=False)
    x = nc.declare_dram_parameter("x", [NB, T, D], f32, isOutput=False)
    ccT = nc.declare_dram_parameter("ccT", [NB, DC, L], f16, isOutput=False)
    condT = nc.declare_dram_parameter("condT", [CF, NB], f16, isOutput=False)
    maskb = nc.declare_dram_parameter("maskb", [NB, L], f32, isOutput=False)
    w_nT = nc.declare_dram_parameter("w_nT", [CF, D], f16, isOutput=False)
    w_cT = nc.declare_dram_parameter("w_cT", [CF, DC], f16, isOutput=False)
    w_qT = nc.declare_dram_parameter("w_qT", [D, D], f16, isOutput=False)
    w_kvT = nc.declare_dram_parameter("w_kvT", [DC, 2 * D], f16, isOutput=False)
    w_oT = nc.declare_dram_parameter("w_oT", [D, D], f16, isOutput=False)
    ind = nc.declare_dram_parameter("ind", [NH, D], f16, isOutput=False)
    indT = nc.declare_dram_parameter("indT", [D, NH], f16, isOutput=False)
    qsc = nc.declare_dram_parameter("qsc", [NH, 1], f32, isOutput=False)
    onesd = nc.declare_dram_parameter("onesd", [128, 1], f16, isOutput=False)
    onesf = nc.declare_dram_parameter("onesf", [1, 1], f32, isOutput=False)
    ksc = nc.declare_dram_parameter("ksc", [NH, 1], f32, isOutput=False)
    out = nc.declare_dram_parameter("out", [NB, T, D], f32, isOutput=True)

    P = 128
    NDC = D // P      # 8 contraction chunks of d / d_cross
    NCF = CF // P     # 6 chunks of cond_f
    NJC = D // P      # 8 chunks of head-dim j (2 heads each)
    NLC = L // P      # 2 chunks of key length

    def mm(ps_, lhsT, rhs, start, stop):
        nc.tensor.matmul(ps_, lhsT, rhs, start=start, stop=stop)

    with tile.TileContext(nc) as tc, ExitStack() as ctx:
        ctx.enter_context(nc.allow_low_precision(
            reason="f32r-typed tiles hold full fp32 bits; PE truncates to FP22"))
        const = ctx.enter_context(tc.tile_pool(name="const", bufs=1))
        acts = ctx.enter_context(tc.tile_pool(name="acts", bufs=1))
        small = ctx.enter_context(tc.tile_pool(name="small", bufs=2))
        ps = ctx.enter_context(tc.tile_pool(name="ps", bufs=1, space="PSUM"))

        def psmm():
            return ps.tile([P, CH], f32, tag="mm", bufs=4, name="mmps")

        # ---- constants ----
        ones = const.tile([P, 1], f16)
        nc.sync.dma_start(out=ones, in_=onesd[:])
        onef = const.tile([1, 1], f32)
        nc.sync.dma_start(out=onef, in_=onesf[:])
        eps_t = const.tile([P, 1], f32)
        nc.vector.memset(eps_t, EPS)
        ind_sb = const.tile([NH, NJC, P], f16)
        nc.sync.dma_start(out=ind_sb, in_=ind.rearrange("h (jc p) -> h jc p", p=P))
        indT_sb = const.tile([P, NJC, NH], f16)
        nc.sync.dma_start(out=indT_sb, in_=indT.rearrange("(jc p) h -> p jc h", p=P))
        qsc_sb = const.tile([NH, 1], f32)
        nc.sync.dma_start(out=qsc_sb, in_=qsc[:])
        ksc_sb = const.tile([NH, 1], f32)
        nc.sync.dma_start(out=ksc_sb, in_=ksc[:])
        mb_sb = const.tile([P, NLC, NB], f32)
        cond_sb = const.tile([P, NCF, NB], f16)
        for b in range(NB):
            nc.sync.dma_start(out=mb_sb[:, :, b],
                              in_=maskb[b].rearrange("(lc p) -> p lc", p=P))
            nc.sync.dma_start(out=cond_sb[:, :, b],
                              in_=condT[:, b].rearrange("(c p) -> p c", p=P))
        s_x = const.tile([P, NDC, NB], f32)
        s_c = const.tile([P, NDC, NB], f32)
        gam = const.tile([P, NLC, NB], f32)  # inv_rms_c per l position

        # ---- stage A: s_x = cond @ w_norm.T + 1, s_c = cond @ w_cnorm.T + 1 ----
        with tc.tile_pool(name="pnorm", bufs=1) as pnorm:
            for wdram, dst in ((w_nT, s_x), (w_cT, s_c)):
                w_sb = pnorm.tile([P, NCF, D], f16, tag="wnorm")
                nc.sync.dma_start(out=w_sb,
                                  in_=wdram.rearrange("(c p) j -> p c j", p=P))
                sps = ps.tile([P, NDC, NB], f32, tag="stat", bufs=2)
                for jc in range(NDC):
                    for c in range(NCF):
                        mm(sps[:, jc, :], w_sb[:, c, jc * P:(jc + 1) * P],
                           cond_sb[:, c, :], start=(c == 0), stop=(c == NCF - 1))
                nc.vector.tensor_scalar_add(dst[:], sps[:], 1.0)

        # ---- stage B: kT (cosine-normalized) and v (+ones col) per batch ----
        kT_sb = []   # [128(j), NJC, L]
        v_sb = []    # [128(l), NLC, NH, 65]
        for b in range(NB):
            kT_sb.append(acts.tile([P, NJC, L], f16, tag=f"kT{b}", name=f"kT{b}"))
            v_sb.append(acts.tile([P, NLC, NH, D_HEAD + 1], f16, tag=f"v{b}", name=f"v{b}"))
        with tc.tile_pool(name="pkv", bufs=1) as pkv:
            wkv_sb = pkv.tile([P, NDC, 2 * D], f16, tag="wkv")
            nc.sync.dma_start(out=wkv_sb,
                              in_=w_kvT.rearrange("(c p) j -> p c j", p=P))
            for b in range(NB):
                kt, vt = kT_sb[b], v_sb[b]
                cc = pkv.tile([P, NDC, L], f16, tag="cc")
                nc.sync.dma_start(out=cc,
                                  in_=ccT[b].rearrange("(c p) l -> p c l", p=P))

                # gamma = rsqrt(mean(cc^2) + eps) from raw cc
                ccsq = pkv.tile([P, NDC, L], f16, tag="ccsq")
                nc.vector.tensor_mul(ccsq[:], cc[:], cc[:])
                msq = ps.tile([1, L], f32, tag="stat", bufs=2)
                for c in range(NDC):
                    mm(msq, ones, ccsq[:, c, :], start=(c == 0), stop=(c == NDC - 1))
                gr = small.tile([1, L], f32, tag="gamr")
                nc.scalar.activation(out=gr, in_=msq, func=Sqrt,
                                     bias=eps_t[:1], scale=1.0 / DC)
                nc.vector.reciprocal(out=gr, in_=gr)
                # broadcast gamma row across partitions via K=1 fp32 matmul
                gps = ps.tile([P, NLC], f32, tag="stat", bufs=2, name="gps")
                for lc in range(NLC):
                    nc.tensor.matmul(gps[:, lc:lc + 1],
                                     gr[0:1, lc * P:(lc + 1) * P],
                                     onef[:],
                                     start=True, stop=True)
                nc.scalar.copy(out=gam[:, :, b], in_=gps)

                # ccq = cc * s_c (in place)
                for c in range(NDC):
                    nc.vector.tensor_scalar_mul(cc[:, c, :], cc[:, c, :],
                                                s_c[:, c, b:b + 1])

                # kT[j, l]
                for jc in range(NJC):
                    kps = ps.tile([P, L], f32, tag="mm", bufs=4)
                    for c in range(NDC):
                        mm(kps, wkv_sb[:, c, jc * P:(jc + 1) * P], cc[:, c, :],
                           start=(c == 0), stop=(c == NDC - 1))
                    nc.scalar.copy(out=kt[:, jc, :], in_=kps)

                # v[l, h, e] * gamma[l], ones col
                for lc in range(NLC):
                    nc.gpsimd.dma_start(out=vt[:, lc, :, D_HEAD],
                                        in_=onesd[:].to_broadcast((P, NH)))
                for lc in range(NLC):
                    for vjc in range(2):
                        vps = psmm()
                        for c in range(NDC):
                            mm(vps, cc[:, c, lc * P:(lc + 1) * P],
                               wkv_sb[:, c, D + vjc * CH:D + (vjc + 1) * CH],
                               start=(c == 0), stop=(c == NDC - 1))
                        nc.vector.tensor_scalar_mul(
                            vt[:, lc, 8 * vjc:8 * (vjc + 1), :D_HEAD],
                            vps.rearrange("p (h e) -> p h e", e=D_HEAD),
                            gam[:, lc, b:b + 1])

                # cosine-normalize k
                ksq = pkv.tile([P, NJC, L], f16, tag="ksq")
                nc.vector.tensor_mul(ksq[:], kt[:], kt[:])
                kss = ps.tile([NH, L], f32, tag="stat", bufs=2)
                for jc in range(NJC):
                    mm(kss, indT_sb[:, jc, :], ksq[:, jc, :],
                       start=(jc == 0), stop=(jc == NJC - 1))
                gkT = small.tile([NH, L], f16, tag="gkT")
                nc.scalar.activation(out=gkT, in_=kss, func=Sqrt,
                                     bias=eps_t[:NH], scale=1.0)
                nc.vector.reciprocal(out=gkT, in_=gkT)
                nc.vector.tensor_scalar_mul(gkT, gkT, ksc_sb)
                for jc in range(NJC):
                    gkb = ps.tile([P, L], f32, tag="mm", bufs=4)
                    mm(gkb, ind_sb[:, jc, :], gkT, start=True, stop=True)
                    nc.vector.tensor_tensor(kt[:, jc, :], kt[:, jc, :], gkb, MULT)

        # ---- stages C/D/E: stream 512-token chunks ----
        with tc.tile_pool(name="pw2", bufs=1) as pw2:
            wq_sb = pw2.tile([P, NDC, D], f16, tag="wq")
            nc.sync.dma_start(out=wq_sb, in_=w_qT.rearrange("(c p) j -> p c j", p=P))
            wo_sb = pw2.tile([P, NJC, D], f16, tag="wo")
            nc.sync.dma_start(out=wo_sb, in_=w_oT.rearrange("(c p) j -> p c j", p=P))

            for chunk in range(NB * (T // CH)):
                b, th = chunk // (T // CH), chunk % (T // CH)
                tsl = slice(th * CH, (th + 1) * CH)
                kt, vt = kT_sb[b], v_sb[b]

                xq = pw2.tile([P, NDC, CH], f16, tag="xq", bufs=1)
                nc.sync.dma_start(
                    out=xq, in_=xT[b].rearrange("(c p) t -> p c t", p=P)[:, :, tsl])
                for c in range(NDC):
                    nc.vector.tensor_scalar_mul(xq[:, c, :], xq[:, c, :],
                                                s_x[:, c, b:b + 1])

                # q projection
                q = pw2.tile([P, NJC, CH], f16, tag="q")
                for jc in range(NJC):
                    qps = psmm()
                    for c in range(NDC):
                        mm(qps, wq_sb[:, c, jc * P:(jc + 1) * P], xq[:, c, :],
                           start=(c == 0), stop=(c == NDC - 1))
                    nc.scalar.copy(out=q[:, jc, :], in_=qps)

                # cosine-normalize q (qsc includes the 1/sqrt(D_HEAD) scale)
                qss = ps.tile([NH, CH], f32, tag="stat", bufs=2)
                for jc in range(NJC):
                    qsq = small.tile([P, CH], f16, tag="qsq")
                    nc.vector.tensor_mul(qsq[:], q[:, jc, :], q[:, jc, :])
                    mm(qss, indT_sb[:, jc, :], qsq,
                       start=(jc == 0), stop=(jc == NJC - 1))
                gqT = small.tile([NH, CH], f16, tag="gqT")
                nc.scalar.activation(out=gqT, in_=qss, func=Sqrt,
                                     bias=eps_t[:NH], scale=1.0)
                nc.vector.reciprocal(out=gqT, in_=gqT)
                nc.vector.tensor_scalar_mul(gqT, gqT, qsc_sb)
                for jc in range(NJC):
                    gqb = psmm()
                    mm(gqb, ind_sb[:, jc, :], gqT, start=True, stop=True)
                    nc.vector.tensor_tensor(q[:, jc, :], q[:, jc, :], gqb, MULT)

                # attention per head
                o = pw2.tile([P, NJC, CH], f16, tag="o")
                den = small.tile([NH, CH], f16, tag="den")
                for h in range(NH):
                    jc, hf = h // 2, h % 2
                    r0, r1 = hf * D_HEAD, (hf + 1) * D_HEAD
                    E = small.tile([P, NLC, CH], f16, tag="E")
                    for lc in range(NLC):
                        scp = psmm()
                        mm(scp, kt[r0:r1, jc, lc * P:(lc + 1) * P],
                           q[r0:r1, jc, :], start=True, stop=True)
                        nc.scalar.activation(out=E[:, lc, :], in_=scp, func=Exp,
                                             bias=mb_sb[:, lc, b:b + 1], scale=1.0)
                    oap = ps.tile([D_HEAD + 1, CH], f32, tag="mm", bufs=4)
                    for lc in range(NLC):
                        mm(oap, vt[:, lc, h, :], E[:, lc, :],
                           start=(lc == 0), stop=(lc == NLC - 1))
                    nc.scalar.copy(out=o[r0:r1, jc, :], in_=oap[:D_HEAD, :])
                    dtmp = small.tile([1, CH], f16, tag="dtmp", name="dtmp")
                    nc.vector.tensor_copy(out=dtmp, in_=oap[D_HEAD:, :])
                    nc.sync.dma_start(out=den[h:h + 1, :], in_=dtmp)

                # divide by softmax denominator
                nc.vector.reciprocal(out=den, in_=den)
                for jc in range(NJC):
                    dbp = psmm()
                    mm(dbp, ind_sb[:, jc, :], den, start=True, stop=True)
                    nc.vector.tensor_tensor(o[:, jc, :], o[:, jc, :], dbp, MULT)

                # out projection + skip
                for t4 in range(CH // P):
                    trow = th * CH + t4 * P
                    xs = small.tile([P, D], f32, tag="xs")
                    nc.sync.dma_start(out=xs, in_=x[b, trow:trow + P, :])
                    os_ = small.tile([P, D], f32, tag="os")
                    for d2 in range(2):
                        ops = psmm()
                        for jc in range(NJC):
                            mm(ops, o[:, jc, t4 * P:(t4 + 1) * P],
                               wo_sb[:, jc, d2 * CH:(d2 + 1) * CH],
                               start=(jc == 0), stop=(jc == NJC - 1))
                        nc.vector.tensor_tensor(os_[:, d2 * CH:(d2 + 1) * CH], ops,
                                                xs[:, d2 * CH:(d2 + 1) * CH], ADD)
                    nc.sync.dma_start(out=out[b, trow:trow + P, :], in_=os_)

    nc.compile()
    return nc


def _prep_inputs(x, cond, crossattn_cond, crossattn_mask, w_norm, w_q, w_cnorm,
                 w_kv, qk_scale, w_o):
    """Shard + lay out the full inputs into 8 per-core input maps."""
    f = np.float32
    h = np.float16
    shared = {
        "w_nT": np.ascontiguousarray(w_norm.T).astype(h),
        "w_cT": np.ascontiguousarray(w_cnorm.T).astype(h),
        "w_qT": np.ascontiguousarray(w_q.T).astype(h),
        "w_kvT": np.ascontiguousarray(w_kv.T).astype(h),
        "w_oT": np.ascontiguousarray(w_o.T).astype(h),
        "ind": np.kron(np.eye(NH, dtype=h), np.ones((1, D_HEAD), dtype=h)),
        "indT": np.kron(np.eye(NH, dtype=h), np.ones((D_HEAD, 1), dtype=h)),
        "qsc": (np.sqrt(qk_scale.astype(f))
                / np.sqrt(np.float32(D_HEAD))).reshape(NH, 1).astype(f),
        "ksc": np.sqrt(qk_scale.astype(f)).reshape(NH, 1).astype(f),
        "onesd": np.ones((128, 1), dtype=h),
        "onesf": np.ones((1, 1), dtype=f),
    }
    in_maps = []
    for c in range(NCORES):
        s = slice(c * NB, (c + 1) * NB)
        xc = np.ascontiguousarray(x[s], dtype=f).reshape(NB, T, D)
        ccc = np.ascontiguousarray(crossattn_cond[s], dtype=f)
        m = {
            "x": xc,
            "xT": np.ascontiguousarray(xc.transpose(0, 2, 1)).astype(h),
            "ccT": np.ascontiguousarray(ccc.transpose(0, 2, 1)).astype(h),
            "condT": np.ascontiguousarray(cond[s].T, dtype=f).astype(h),
            "maskb": np.where(crossattn_mask[s], f(0.0), f(MASK_NEG)).astype(f),
        }
        m.update(shared)
        in_maps.append(m)
    return in_maps


def _run(inputs, trace=False):
    from concourse.bass_utils import run_bass_kernel_spmd

    if "nc" not in _cached:
        _cached["nc"] = _build_nc()
    nc = _cached["nc"]
    in_maps = _prep_inputs(**inputs)
    res = run_bass_kernel_spmd(nc, in_maps, core_ids=list(range(NCORES)),
                               trace=trace)
    outs = np.concatenate([r["out"] for r in res.results], axis=0)
    return outs.reshape(N, H, W, D), res


def kernel(**inputs):
    out, _ = _run(inputs, trace=False)
    return out



# revision 8
# speedup vs baseline: 1.2151x; 1.2151x over previous
"""CrossAttentionBlock Trainium2 kernel — data-parallel over batch across 8 cores.

Full inputs in, full outputs out. Each core handles 2 of the 16 batch
elements; weights are replicated. No collectives.

Math notes (vs the jax reference):
- AdaRMSNorm on x: xn = x * s_x[d] * inv_rms_x[t].  inv_rms_x is a positive
  per-token scalar; q = xn @ w_q.T is later cosine-normalized per head, so
  inv_rms_x cancels (up to a negligible eps perturbation) and is skipped.
- AdaRMSNorm on crossattn_cond: the inv_rms_c factor cancels for k (cosine
  normalized) but NOT for v, so it is folded into v only.
- The boolean key mask is applied MULTIPLICATIVELY on the v side: v rows
  (and the appended all-ones denominator column) of masked keys are zeroed,
  which removes them from both the softmax numerator and denominator —
  exactly equivalent to the additive -inf mask.  exp() then needs no bias,
  so each head's softmax numerator is one [128, 2*CH] activation.
- Cosine-sim scores are bounded (|score| <= qk_scale/sqrt(D_HEAD)), so
  softmax runs without max-subtraction.
- All rsqrt/reciprocal are computed as exp(a*ln(x)+b) on the scalar engine:
  ln and exp live in one activation-table set, so the LUT never reloads.
- Softmax denominator comes from an extra all-ones column appended to v; the
  per-(head, token) row is DMA'd out of PSUM and broadcast across partitions
  with a K=16 indicator matmul.

Every DRAM input is host-preswizzled so each DMA is contiguous per
partition (the baseline lost ~100us to 2-byte strided descriptors).
"""

import numpy as np

D_HEAD = 64
EPS = 1e-6
N, H, W, D = 16, 32, 32, 1024
L, DC, CF = 256, 1024, 768
NH = D // D_HEAD  # 16
NCORES = 8
NB = N // NCORES  # 2 batch elements per core
T = H * W  # 1024 tokens per batch element
CH = 512  # token chunk
NCH = T // CH

_cached = {}


def _build_nc():
    from contextlib import ExitStack

    import concourse.mybir as mybir
    import concourse.tile as tile
    from concourse import bacc

    f32 = mybir.dt.float32
    f16 = mybir.dt.float16
    Exp = mybir.ActivationFunctionType.Exp
    Ln = mybir.ActivationFunctionType.Ln
    MULT = mybir.AluOpType.mult
    ADD = mybir.AluOpType.add

    nc = bacc.Bacc(None, target_bir_lowering=False)

    P = 128
    NDC = D // P      # 8 contraction chunks of d / d_cross
    NCF = CF // P     # 6 chunks of cond_f
    NJC = D // P      # 8 chunks of head-dim j (2 heads each)
    NLC = L // P      # 2 chunks of key length

    # ---- DRAM parameters (all host-preswizzled, partition-major) ----
    xT = nc.declare_dram_parameter("xT", [NB, P, NDC, T], f16, isOutput=False)
    x = nc.declare_dram_parameter("x", [NB, T, D], f32, isOutput=False)
    ccT = nc.declare_dram_parameter("ccT", [NB, P, NDC, L], f16, isOutput=False)
    condP = nc.declare_dram_parameter("condP", [P, NCF, NB], f16, isOutput=False)
    gateP = nc.declare_dram_parameter("gateP", [P, NLC, NB], f32, isOutput=False)
    w_n = nc.declare_dram_parameter("w_n", [P, NCF, D], f16, isOutput=False)
    w_c = nc.declare_dram_parameter("w_c", [P, NCF, DC], f16, isOutput=False)
    w_q = nc.declare_dram_parameter("w_q", [P, NDC, D], f16, isOutput=False)
    w_kv = nc.declare_dram_parameter("w_kv", [P, NDC, 2 * D], f16, isOutput=False)
    w_o = nc.declare_dram_parameter("w_o", [P, NJC, D], f16, isOutput=False)
    ind = nc.declare_dram_parameter("ind", [NH, NJC, P], f16, isOutput=False)
    indT = nc.declare_dram_parameter("indT", [P, NJC, NH], f16, isOutput=False)
    lnqsc = nc.declare_dram_parameter("lnqsc", [NH, 1], f32, isOutput=False)
    lnksc = nc.declare_dram_parameter("lnksc", [NH, 1], f32, isOutput=False)
    onesd = nc.declare_dram_parameter("onesd", [P, 1], f16, isOutput=False)
    onesf = nc.declare_dram_parameter("onesf", [1, 1], f32, isOutput=False)
    out = nc.declare_dram_parameter("out", [NB, T, D], f32, isOutput=True)

    def mm(ps_, lhsT, rhs, start, stop):
        nc.tensor.matmul(ps_, lhsT, rhs, start=start, stop=stop)

    with tile.TileContext(nc) as tc, ExitStack() as ctx:
        ctx.enter_context(nc.allow_low_precision(
            reason="f16 matmuls; fp32 accumulate in PSUM"))
        const = ctx.enter_context(tc.tile_pool(name="const", bufs=1))
        acts = ctx.enter_context(tc.tile_pool(name="acts", bufs=1))
        small = ctx.enter_context(tc.tile_pool(name="small", bufs=2))
        ps = ctx.enter_context(tc.tile_pool(name="ps", bufs=1, space="PSUM"))

        def psmm():
            # shared [128, 512] fp32 psum ring: projections, oap, broadcasts
            return ps.tile([P, CH], f32, tag="mm", bufs=3, name="mmps")

        # ---- constants ----
        ones = const.tile([P, 1], f16)
        nc.sync.dma_start(out=ones, in_=onesd[:])
        onef = const.tile([1, 1], f32)
        nc.sync.dma_start(out=onef, in_=onesf[:])
        eps_t = const.tile([P, 1], f32)
        nc.vector.memset(eps_t, EPS)
        ind_sb = const.tile([NH, NJC, P], f16)
        nc.sync.dma_start(out=ind_sb, in_=ind[:])
        indT_sb = const.tile([P, NJC, NH], f16)
        nc.sync.dma_start(out=indT_sb, in_=indT[:])
        lnqsc_sb = const.tile([NH, 1], f32)
        nc.sync.dma_start(out=lnqsc_sb, in_=lnqsc[:])
        lnksc_sb = const.tile([NH, 1], f32)
        nc.sync.dma_start(out=lnksc_sb, in_=lnksc[:])
        gate_sb = const.tile([P, NLC, NB], f32)
        nc.sync.dma_start(out=gate_sb, in_=gateP[:])
        cond_sb = const.tile([P, NCF, NB], f16)
        nc.sync.dma_start(out=cond_sb, in_=condP[:])
        s_x = const.tile([P, NDC, NB], f32)
        s_c = const.tile([P, NDC, NB], f32)
        gamg = const.tile([P, NLC, NB], f32)  # gate * inv_rms_c per l position

        # ---- stage A: s_x = cond @ w_norm.T + 1, s_c = cond @ w_cnorm.T + 1 ----
        # one [128, 512] psum tile holds all 8 jc column-pairs for both stats
        with tc.tile_pool(name="pnorm", bufs=1) as pnorm:
            sps = psmm()
            for wi, (wdram, dst) in enumerate(((w_n, s_x), (w_c, s_c))):
                w_sb = pnorm.tile([P, NCF, D], f16, tag="wnorm")
                nc.sync.dma_start(out=w_sb, in_=wdram[:])
                for jc in range(NDC):
                    col = wi * 256 + jc * 2
                    for c in range(NCF):
                        mm(sps[:, col:col + 2], w_sb[:, c, jc * P:(jc + 1) * P],
                           cond_sb[:, c, :], start=(c == 0), stop=(c == NCF - 1))
            for wi, dst in enumerate((s_x, s_c)):
                for jc in range(NDC):
                    col = wi * 256 + jc * 2
                    nc.vector.tensor_scalar_add(dst[:, jc, :], sps[:, col:col + 2],
                                                1.0)

        # ---- stage B: kT (cosine-normalized) and v (+gated ones col) ----
        kT_sb = []   # [128(j), NJC, L]
        v_sb = []    # [128(l), NLC, NH, 65]
        for b in range(NB):
            kT_sb.append(acts.tile([P, NJC, L], f16, tag=f"kT{b}", name=f"kT{b}"))
            v_sb.append(acts.tile([P, NLC, NH, D_HEAD + 1], f16, tag=f"v{b}",
                                  name=f"v{b}"))
        with tc.tile_pool(name="pkv", bufs=1) as pkv:
            wkv_sb = pkv.tile([P, NDC, 2 * D], f16, tag="wkv")
            nc.sync.dma_start(out=wkv_sb, in_=w_kv[:])
            for b in range(NB):
                kt, vt = kT_sb[b], v_sb[b]
                cc = pkv.tile([P, NDC, L], f16, tag="cc", bufs=2)
                nc.sync.dma_start(out=cc, in_=ccT[b])

                # gamma = rsqrt(mean(cc^2) + eps) from raw cc
                ccsq = pkv.tile([P, NDC, L], f16, tag="ccsq", bufs=2)
                nc.vector.tensor_mul(ccsq[:], cc[:], cc[:])
                st = ps.tile([NH, CH], f32, tag="stat", bufs=1, name="stat")
                for c in range(NDC):
                    mm(st[:1, :L], ones, ccsq[:, c, :], start=(c == 0),
                       stop=(c == NDC - 1))
                grl = small.tile([1, L], f32, tag="grl")
                nc.scalar.activation(out=grl, in_=st[:1, :L], func=Ln,
                                     bias=eps_t[:1], scale=1.0 / DC)
                gr = small.tile([1, L], f32, tag="gamr")
                nc.scalar.activation(out=gr, in_=grl, func=Exp, scale=-0.5)
                # broadcast gamma row across partitions via K=1 fp32 matmul
                gps = psmm()
                for lc in range(NLC):
                    nc.tensor.matmul(gps[:, lc:lc + 1],
                                     gr[0:1, lc * P:(lc + 1) * P],
                                     onef[:], start=True, stop=True)
                # gamg = gamma * mask gate
                nc.vector.tensor_tensor(gamg[:, :, b], gps[:, :NLC],
                                        gate_sb[:, :, b], MULT)

                # ccq = cc * s_c (in place)
                for c in range(NDC):
                    nc.vector.tensor_scalar_mul(cc[:, c, :], cc[:, c, :],
                                                s_c[:, c, b:b + 1])

                # kT[j, l]
                for jc in range(NJC):
                    kps = psmm()
                    for c in range(NDC):
                        mm(kps[:, :L], wkv_sb[:, c, jc * P:(jc + 1) * P],
                           cc[:, c, :], start=(c == 0), stop=(c == NDC - 1))
                    nc.scalar.copy(out=kt[:, jc, :], in_=kps[:, :L])

                # v[l, h, e] * gamg[l]; ones col * gate[l]
                for lc in range(NLC):
                    nc.vector.memset(vt[:, lc, :, D_HEAD], 1.0)
                    nc.vector.tensor_scalar_mul(vt[:, lc, :, D_HEAD],
                                                vt[:, lc, :, D_HEAD],
                                                gate_sb[:, lc, b:b + 1])
                for lc in range(NLC):
                    for vjc in range(2):
                        vps = psmm()
                        for c in range(NDC):
                            mm(vps, cc[:, c, lc * P:(lc + 1) * P],
                               wkv_sb[:, c, D + vjc * CH:D + (vjc + 1) * CH],
                               start=(c == 0), stop=(c == NDC - 1))
                        nc.vector.tensor_scalar_mul(
                            vt[:, lc, 8 * vjc:8 * (vjc + 1), :D_HEAD],
                            vps.rearrange("p (h e) -> p h e", e=D_HEAD),
                            gamg[:, lc, b:b + 1])

                # cosine-normalize k:  k *= exp(-0.5*ln(|k|^2+eps) + ln(ksc))
                ksq = pkv.tile([P, NJC, L], f16, tag="ksq", bufs=2)
                nc.vector.tensor_mul(ksq[:], kt[:], kt[:])
                st2 = ps.tile([NH, CH], f32, tag="stat", bufs=1, name="stat")
                for jc in range(NJC):
                    mm(st2[:, :L], indT_sb[:, jc, :], ksq[:, jc, :],
                       start=(jc == 0), stop=(jc == NJC - 1))
                gkl = small.tile([NH, L], f32, tag="gkl")
                nc.scalar.activation(out=gkl, in_=st2[:, :L], func=Ln,
                                     bias=eps_t[:NH], scale=1.0)
                gkT = small.tile([NH, L], f16, tag="gkT")
                nc.scalar.activation(out=gkT, in_=gkl, func=Exp,
                                     bias=lnksc_sb, scale=-0.5)
                for jc in range(NJC):
                    gkb = psmm()
                    mm(gkb[:, :L], ind_sb[:, jc, :], gkT, start=True, stop=True)
                    nc.vector.tensor_tensor(kt[:, jc, :], kt[:, jc, :],
                                            gkb[:, :L], MULT)

        # ---- stages C/D/E: stream 512-token chunks ----
        with tc.tile_pool(name="pw2", bufs=1) as pw2:
            wq_sb = pw2.tile([P, NDC, D], f16, tag="wq")
            nc.sync.dma_start(out=wq_sb, in_=w_q[:])
            wo_sb = pw2.tile([P, NJC, D], f16, tag="wo")
            nc.sync.dma_start(out=wo_sb, in_=w_o[:])

            xq_b = {}
            for b in range(NB):
                # whole-batch transposed x, one contiguous DMA; scale by s_x
                xq = pw2.tile([P, NDC, T], f16, tag="xq", bufs=2)
                nc.sync.dma_start(out=xq, in_=xT[b])
                for c in range(NDC):
                    nc.vector.tensor_scalar_mul(xq[:, c, :], xq[:, c, :],
                                                s_x[:, c, b:b + 1])
                xq_b[b] = xq

            for chunk in range(NB * NCH):
                b, th = chunk // NCH, chunk % NCH
                tsl = slice(th * CH, (th + 1) * CH)
                kt, vt = kT_sb[b], v_sb[b]
                xq = xq_b[b]

                # q projection
                q = pw2.tile([P, NJC, CH], f16, tag="q", bufs=2)
                for jc in range(NJC):
                    qps = psmm()
                    for c in range(NDC):
                        mm(qps, wq_sb[:, c, jc * P:(jc + 1) * P], xq[:, c, tsl],
                           start=(c == 0), stop=(c == NDC - 1))
                    nc.scalar.copy(out=q[:, jc, :], in_=qps)

                # cosine-normalize q (lnqsc includes the 1/sqrt(D_HEAD) scale)
                qst = ps.tile([NH, CH], f32, tag="stat", bufs=1, name="stat")
                for jc in range(NJC):
                    qsq = small.tile([P, CH], f16, tag="qsq")
                    nc.vector.tensor_mul(qsq[:], q[:, jc, :], q[:, jc, :])
                    mm(qst, indT_sb[:, jc, :], qsq,
                       start=(jc == 0), stop=(jc == NJC - 1))
                gql = small.tile([NH, CH], f32, tag="gql")
                nc.scalar.activation(out=gql, in_=qst, func=Ln,
                                     bias=eps_t[:NH], scale=1.0)
                gqT = small.tile([NH, CH], f16, tag="gqT")
                nc.scalar.activation(out=gqT, in_=gql, func=Exp,
                                     bias=lnqsc_sb, scale=-0.5)
                for jc in range(NJC):
                    gqb = psmm()
                    mm(gqb, ind_sb[:, jc, :], gqT, start=True, stop=True)
                    nc.vector.tensor_tensor(q[:, jc, :], q[:, jc, :], gqb, MULT)

                # attention per head; masked keys contribute 0 via gated v
                o = pw2.tile([P, NJC, CH], f16, tag="o", bufs=2)
                denf = small.tile([NH, CH], f16, tag="denf")
                for h in range(NH):
                    jc, hf = h // 2, h % 2
                    r0, r1 = hf * D_HEAD, (hf + 1) * D_HEAD
                    sc = ps.tile([P, NLC, CH], f32, tag="sc", bufs=2, name="sc")
                    for lc in range(NLC):
                        mm(sc[:, lc, :], kt[r0:r1, jc, lc * P:(lc + 1) * P],
                           q[r0:r1, jc, :], start=True, stop=True)
                    E = small.tile([P, NLC, CH], f16, tag="E")
                    nc.scalar.activation(out=E[:], in_=sc[:], func=Exp,
                                         scale=1.0)
                    oap = psmm()
                    for lc in range(NLC):
                        mm(oap[:D_HEAD + 1, :], vt[:, lc, h, :], E[:, lc, :],
                           start=(lc == 0), stop=(lc == NLC - 1))
                    dtmp = small.tile([1, CH], f16, tag="dtmp", bufs=4,
                                      name="dtmp")
                    if h % 2 == 0:
                        nc.scalar.copy(out=o[r0:r1, jc, :], in_=oap[:D_HEAD, :])
                        nc.vector.tensor_copy(out=dtmp,
                                              in_=oap[D_HEAD:D_HEAD + 1, :])
                    else:
                        nc.vector.tensor_copy(out=o[r0:r1, jc, :],
                                              in_=oap[:D_HEAD, :])
                        nc.scalar.copy(out=dtmp,
                                       in_=oap[D_HEAD:D_HEAD + 1, :])
                    nc.sync.dma_start(out=denf[h:h + 1, :], in_=dtmp)

                # divide by softmax denominator: 1/x = exp(-ln(x))
                dnl = small.tile([NH, CH], f32, tag="dnl")
                nc.scalar.activation(out=dnl, in_=denf, func=Ln, scale=1.0)
                denr = small.tile([NH, CH], f16, tag="denr")
                nc.scalar.activation(out=denr, in_=dnl, func=Exp, scale=-1.0)
                for jc in range(NJC):
                    dbp = psmm()
                    mm(dbp, ind_sb[:, jc, :], denr, start=True, stop=True)
                    nc.vector.tensor_tensor(o[:, jc, :], o[:, jc, :], dbp, MULT)

                # out projection + skip
                for t4 in range(CH // P):
                    trow = th * CH + t4 * P
                    xs = small.tile([P, D], f32, tag="xs")
                    nc.sync.dma_start(out=xs, in_=x[b, trow:trow + P, :])
                    os_ = small.tile([P, D], f32, tag="os")
                    for d2 in range(2):
                        ops = psmm()
                        for jc in range(NJC):
                            mm(ops, o[:, jc, t4 * P:(t4 + 1) * P],
                               wo_sb[:, jc, d2 * CH:(d2 + 1) * CH],
                               start=(jc == 0), stop=(jc == NJC - 1))
                        nc.vector.tensor_tensor(os_[:, d2 * CH:(d2 + 1) * CH],
                                                ops, xs[:, d2 * CH:(d2 + 1) * CH],
                                                ADD)
                    nc.sync.dma_start(out=out[b, trow:trow + P, :], in_=os_)

    nc.compile()
    return nc


def _swizzle_w(wT, ncols):
    # [K, J] -> [128, K//128, J] partition-major
    K, J = wT.shape
    return np.ascontiguousarray(
        wT.reshape(K // 128, 128, J).transpose(1, 0, 2)).astype(np.float16)


def _prep_inputs(x, cond, crossattn_cond, crossattn_mask, w_norm, w_q, w_cnorm,
                 w_kv, qk_scale, w_o):
    """Shard + lay out the full inputs into 8 per-core input maps."""
    f = np.float32
    h = np.float16
    P = 128
    NDC = D // P
    sqc = np.sqrt(qk_scale.astype(f))
    shared = {
        "w_n": _swizzle_w(np.ascontiguousarray(w_norm.T), CF),
        "w_c": _swizzle_w(np.ascontiguousarray(w_cnorm.T), CF),
        "w_q": _swizzle_w(np.ascontiguousarray(w_q.T), D),
        "w_kv": _swizzle_w(np.ascontiguousarray(w_kv.T), DC),
        "w_o": _swizzle_w(np.ascontiguousarray(w_o.T), D),
        "ind": np.kron(np.eye(NH, dtype=h),
                       np.ones((1, D_HEAD), dtype=h)).reshape(NH, NDC, P),
        "indT": np.ascontiguousarray(
            np.kron(np.eye(NH, dtype=h), np.ones((D_HEAD, 1), dtype=h))
            .reshape(NDC, P, NH).transpose(1, 0, 2)),
        "lnqsc": np.log(sqc / np.sqrt(f(D_HEAD))).reshape(NH, 1).astype(f),
        "lnksc": np.log(sqc).reshape(NH, 1).astype(f),
        "onesd": np.ones((P, 1), dtype=h),
        "onesf": np.ones((1, 1), dtype=f),
    }
    in_maps = []
    for c in range(NCORES):
        s = slice(c * NB, (c + 1) * NB)
        xc = np.ascontiguousarray(x[s], dtype=f).reshape(NB, T, D)
        xt = xc.transpose(0, 2, 1).reshape(NB, NDC, P, T).transpose(0, 2, 1, 3)
        ccc = np.ascontiguousarray(crossattn_cond[s], dtype=f)
        cct = ccc.transpose(0, 2, 1).reshape(NB, NDC, P, L).transpose(0, 2, 1, 3)
        m = {
            "x": xc,
            "xT": np.ascontiguousarray(xt).astype(h),
            "ccT": np.ascontiguousarray(cct).astype(h),
            "condP": np.ascontiguousarray(
                cond[s].T.reshape(CF // P, P, NB).transpose(1, 0, 2)).astype(h),
            "gateP": np.ascontiguousarray(
                crossattn_mask[s].astype(f).T.reshape(L // P, P, NB)
                .transpose(1, 0, 2)),
        }
        m.update(shared)
        in_maps.append(m)
    return in_maps


def _run(inputs, trace=False):
    from concourse.bass_utils import run_bass_kernel_spmd

    if "nc" not in _cached:
        _cached["nc"] = _build_nc()
    nc = _cached["nc"]
    in_maps = _prep_inputs(**inputs)
    res = run_bass_kernel_spmd(nc, in_maps, core_ids=list(range(NCORES)),
                               trace=trace)
    outs = np.concatenate([r["out"] for r in res.results], axis=0)
    return outs.reshape(N, H, W, D), res


def kernel(**inputs):
    out, _ = _run(inputs, trace=False)
    return out


# revision 11
# speedup vs baseline: 1.4445x; 1.1887x over previous
"""CrossAttentionBlock Trainium2 kernel — data-parallel over batch across 8 cores.

Full inputs in, full outputs out. Each core handles 2 of the 16 batch
elements; weights are replicated. No collectives.

Math notes (vs the jax reference):
- AdaRMSNorm on x: xn = x * s_x[d] * inv_rms_x[t].  inv_rms_x is a positive
  per-token scalar; q = xn @ w_q.T is later cosine-normalized per head, so
  inv_rms_x cancels (up to a negligible eps perturbation) and is skipped.
- AdaRMSNorm on crossattn_cond: the inv_rms_c factor cancels for k (cosine
  normalized) but NOT for v, so it is folded into v only.
- The boolean key mask is applied MULTIPLICATIVELY on the v side: v rows
  (and the appended all-ones denominator column) of masked keys are zeroed,
  which removes them from both the softmax numerator and denominator —
  exactly equivalent to the additive -inf mask.  exp() then needs no bias,
  so each head's softmax numerator is one [128, 2*CH] activation.
- Cosine-sim scores are bounded (|score| <= qk_scale/sqrt(D_HEAD)), so
  softmax runs without max-subtraction.
- All rsqrt/reciprocal are computed as exp(a*ln(x)+b) on the scalar engine:
  ln and exp live in one activation-table set, so the LUT never reloads.
- Softmax denominator comes from an extra all-ones column appended to v; the
  per-(head, token) row is DMA'd out of PSUM and broadcast across partitions
  with a K=16 indicator matmul.

Every DRAM input is host-preswizzled so each DMA is contiguous per
partition (the baseline lost ~100us to 2-byte strided descriptors).
"""

import numpy as np

D_HEAD = 64
EPS = 1e-6
N, H, W, D = 16, 32, 32, 1024
L, DC, CF = 256, 1024, 768
NH = D // D_HEAD  # 16
NCORES = 8
NB = N // NCORES  # 2 batch elements per core
T = H * W  # 1024 tokens per batch element
CH = 512  # token chunk
NCH = T // CH

_cached = {}


def _build_nc():
    from contextlib import ExitStack

    import concourse.mybir as mybir
    import concourse.tile as tile
    from concourse import bacc

    f32 = mybir.dt.float32
    f16 = mybir.dt.float16
    Exp = mybir.ActivationFunctionType.Exp
    Ln = mybir.ActivationFunctionType.Ln
    MULT = mybir.AluOpType.mult
    ADD = mybir.AluOpType.add

    # Both Ln and Exp live in the "natural_log_exp_and_others" activation
    # table set, but the default chooser assigns Exp to "exp_and_others",
    # reloading the LUT on every Ln<->Exp alternation (24 loads/kernel).
    # Restrict every other set's claim on the functions this kernel uses so
    # one table load covers the whole kernel.
    import concourse.bacc as _bacc_mod
    _orig_gat = _bacc_mod.get_activation_tables

    def _gat(arch):
        tabs = dict(_orig_gat(arch))
        keep = "natural_log_exp_and_others"
        if keep in tabs:
            kf = tabs[keep]
            tabs = {k: (v if k == keep else (v - kf)) for k, v in tabs.items()}
        return tabs

    _bacc_mod.get_activation_tables = _gat

    nc = bacc.Bacc(None, target_bir_lowering=False)

    P = 128
    NDC = D // P      # 8 contraction chunks of d / d_cross
    NCF = CF // P     # 6 chunks of cond_f
    NJC = D // P      # 8 chunks of head-dim j (2 heads each)
    NLC = L // P      # 2 chunks of key length

    # ---- DRAM parameters (all host-preswizzled, partition-major) ----
    xT = nc.declare_dram_parameter("xT", [NB, P, NDC, T], f16, isOutput=False)
    x = nc.declare_dram_parameter("x", [NB, T, D], f32, isOutput=False)
    ccT = nc.declare_dram_parameter("ccT", [NB, P, NDC, L], f16, isOutput=False)
    condP = nc.declare_dram_parameter("condP", [P, NCF, NB], f16, isOutput=False)
    gateP = nc.declare_dram_parameter("gateP", [P, NLC, NB], f32, isOutput=False)
    w_n = nc.declare_dram_parameter("w_n", [P, NCF, D], f16, isOutput=False)
    w_c = nc.declare_dram_parameter("w_c", [P, NCF, DC], f16, isOutput=False)
    w_q = nc.declare_dram_parameter("w_q", [P, NDC, D], f16, isOutput=False)
    w_kv = nc.declare_dram_parameter("w_kv", [P, NDC, 2 * D], f16, isOutput=False)
    w_o = nc.declare_dram_parameter("w_o", [P, NJC, D], f16, isOutput=False)
    ind = nc.declare_dram_parameter("ind", [NH, NJC, P], f16, isOutput=False)
    indT = nc.declare_dram_parameter("indT", [P, NJC, NH], f16, isOutput=False)
    lnqsc = nc.declare_dram_parameter("lnqsc", [NH, 1], f32, isOutput=False)
    lnksc = nc.declare_dram_parameter("lnksc", [NH, 1], f32, isOutput=False)
    onesd = nc.declare_dram_parameter("onesd", [P, 1], f16, isOutput=False)
    onesf = nc.declare_dram_parameter("onesf", [1, 1], f32, isOutput=False)
    out = nc.declare_dram_parameter("out", [NB, T, D], f32, isOutput=True)

    def mm(ps_, lhsT, rhs, start, stop):
        nc.tensor.matmul(ps_, lhsT, rhs, start=start, stop=stop)

    with tile.TileContext(nc) as tc, ExitStack() as ctx:
        ctx.enter_context(nc.allow_low_precision(
            reason="f16 matmuls; fp32 accumulate in PSUM"))
        const = ctx.enter_context(tc.tile_pool(name="const", bufs=1))
        acts = ctx.enter_context(tc.tile_pool(name="acts", bufs=1))
        small = ctx.enter_context(tc.tile_pool(name="small", bufs=2))
        ps = ctx.enter_context(tc.tile_pool(name="ps", bufs=1, space="PSUM"))

        def psmm():
            # shared [128, 512] fp32 psum ring: projections, oap, broadcasts
            return ps.tile([P, CH], f32, tag="mm", bufs=3, name="mmps")

        # ---- constants ----
        ones = const.tile([P, 1], f16)
        nc.sync.dma_start(out=ones, in_=onesd[:])
        onef = const.tile([1, 1], f32)
        nc.sync.dma_start(out=onef, in_=onesf[:])
        eps_t = const.tile([P, 1], f32)
        nc.vector.memset(eps_t, EPS)
        ind_sb = const.tile([NH, NJC, P], f16)
        nc.sync.dma_start(out=ind_sb, in_=ind[:])
        indT_sb = const.tile([P, NJC, NH], f16)
        nc.sync.dma_start(out=indT_sb, in_=indT[:])
        lnqsc_sb = const.tile([NH, 1], f32)
        nc.sync.dma_start(out=lnqsc_sb, in_=lnqsc[:])
        lnksc_sb = const.tile([NH, 1], f32)
        nc.sync.dma_start(out=lnksc_sb, in_=lnksc[:])
        gate_sb = const.tile([P, NLC, NB], f32)
        nc.sync.dma_start(out=gate_sb, in_=gateP[:])
        cond_sb = const.tile([P, NCF, NB], f16)
        nc.sync.dma_start(out=cond_sb, in_=condP[:])
        s_x = const.tile([P, NDC, NB], f32)
        s_c = const.tile([P, NDC, NB], f32)
        gamg = const.tile([P, NLC, NB], f32)  # gate * inv_rms_c per l position

        # ---- stage A: s_x = cond @ w_norm.T + 1, s_c = cond @ w_cnorm.T + 1 ----
        # one [128, 512] psum tile holds all 8 jc column-pairs for both stats
        with tc.tile_pool(name="pnorm", bufs=1) as pnorm:
            sps = psmm()
            for wi, (wdram, dst) in enumerate(((w_n, s_x), (w_c, s_c))):
                w_sb = pnorm.tile([P, NCF, D], f16, tag="wnorm")
                nc.sync.dma_start(out=w_sb, in_=wdram[:])
                for jc in range(NDC):
                    col = wi * 256 + jc * 2
                    for c in range(NCF):
                        mm(sps[:, col:col + 2], w_sb[:, c, jc * P:(jc + 1) * P],
                           cond_sb[:, c, :], start=(c == 0), stop=(c == NCF - 1))
            for wi, dst in enumerate((s_x, s_c)):
                for jc in range(NDC):
                    col = wi * 256 + jc * 2
                    nc.vector.tensor_scalar_add(dst[:, jc, :], sps[:, col:col + 2],
                                                1.0)

        # ---- stage B: kT (cosine-normalized) and v (+gated ones col) ----
        kT_sb = []   # [128(j), NJC, L]
        v_sb = []    # [128(l), NLC, NH, 65]
        for b in range(NB):
            kT_sb.append(acts.tile([P, NJC, L], f16, tag=f"kT{b}", name=f"kT{b}"))
            v_sb.append(acts.tile([P, NLC, NH, D_HEAD + 1], f16, tag=f"v{b}",
                                  name=f"v{b}"))
        with tc.tile_pool(name="pkv", bufs=1) as pkv:
            wkv_sb = pkv.tile([P, NDC, 2 * D], f16, tag="wkv")
            nc.sync.dma_start(out=wkv_sb, in_=w_kv[:])
            for b in range(NB):
                kt, vt = kT_sb[b], v_sb[b]
                cc = pkv.tile([P, NDC, L], f16, tag="cc", bufs=2)
                nc.sync.dma_start(out=cc, in_=ccT[b])

                # gamma = rsqrt(mean(cc^2) + eps) from raw cc
                ccsq = pkv.tile([P, NDC, L], f16, tag="ccsq", bufs=2)
                nc.vector.tensor_mul(ccsq[:], cc[:], cc[:])
                st = ps.tile([NH, CH], f32, tag="stat", bufs=1, name="stat")
                for c in range(NDC):
                    mm(st[:1, :L], ones, ccsq[:, c, :], start=(c == 0),
                       stop=(c == NDC - 1))
                grl = small.tile([1, L], f32, tag="grl")
                nc.scalar.activation(out=grl, in_=st[:1, :L], func=Ln,
                                     bias=eps_t[:1], scale=1.0 / DC)
                gr = small.tile([1, L], f32, tag="gamr")
                nc.scalar.activation(out=gr, in_=grl, func=Exp, scale=-0.5)
                # broadcast gamma row across partitions via K=1 fp32 matmul
                gps = psmm()
                for lc in range(NLC):
                    nc.tensor.matmul(gps[:, lc:lc + 1],
                                     gr[0:1, lc * P:(lc + 1) * P],
                                     onef[:], start=True, stop=True)
                # gamg = gamma * mask gate
                nc.vector.tensor_tensor(gamg[:, :, b], gps[:, :NLC],
                                        gate_sb[:, :, b], MULT)

                # ccq = cc * s_c (in place)
                for c in range(NDC):
                    nc.vector.tensor_scalar_mul(cc[:, c, :], cc[:, c, :],
                                                s_c[:, c, b:b + 1])

                # kT[j, l]
                for jc in range(NJC):
                    kps = psmm()
                    for c in range(NDC):
                        mm(kps[:, :L], wkv_sb[:, c, jc * P:(jc + 1) * P],
                           cc[:, c, :], start=(c == 0), stop=(c == NDC - 1))
                    nc.scalar.copy(out=kt[:, jc, :], in_=kps[:, :L])

                # v[l, h, e] * gamg[l]; ones col * gate[l]
                for lc in range(NLC):
                    nc.vector.memset(vt[:, lc, :, D_HEAD], 1.0)
                    nc.vector.tensor_scalar_mul(vt[:, lc, :, D_HEAD],
                                                vt[:, lc, :, D_HEAD],
                                                gate_sb[:, lc, b:b + 1])
                for lc in range(NLC):
                    for vjc in range(2):
                        vps = psmm()
                        for c in range(NDC):
                            mm(vps, cc[:, c, lc * P:(lc + 1) * P],
                               wkv_sb[:, c, D + vjc * CH:D + (vjc + 1) * CH],
                               start=(c == 0), stop=(c == NDC - 1))
                        nc.vector.tensor_scalar_mul(
                            vt[:, lc, 8 * vjc:8 * (vjc + 1), :D_HEAD],
                            vps.rearrange("p (h e) -> p h e", e=D_HEAD),
                            gamg[:, lc, b:b + 1])

                # cosine-normalize k:  k *= exp(-0.5*ln(|k|^2+eps) + ln(ksc))
                ksq = pkv.tile([P, NJC, L], f16, tag="ksq", bufs=2)
                nc.vector.tensor_mul(ksq[:], kt[:], kt[:])
                st2 = ps.tile([NH, CH], f32, tag="stat", bufs=1, name="stat")
                for jc in range(NJC):
                    mm(st2[:, :L], indT_sb[:, jc, :], ksq[:, jc, :],
                       start=(jc == 0), stop=(jc == NJC - 1))
                gkl = small.tile([NH, L], f32, tag="gkl")
                nc.scalar.activation(out=gkl, in_=st2[:, :L], func=Ln,
                                     bias=eps_t[:NH], scale=1.0)
                gkT = small.tile([NH, L], f16, tag="gkT")
                nc.scalar.activation(out=gkT, in_=gkl, func=Exp,
                                     bias=lnksc_sb, scale=-0.5)
                for jc in range(NJC):
                    gkb = psmm()
                    mm(gkb[:, :L], ind_sb[:, jc, :], gkT, start=True, stop=True)
                    nc.vector.tensor_tensor(kt[:, jc, :], kt[:, jc, :],
                                            gkb[:, :L], MULT)

        # ---- stages C/D/E: stream 512-token chunks, software-pipelined ----
        # Engine instruction queues are FIFO, so issue order IS the tensor
        # schedule: interleave chunk c+1's q-projection and chunk c-1's
        # out-projection into chunk c's (scalar-paced) attention head loop
        # so the PE never idles and HAM stays at full clock.
        with tc.tile_pool(name="pw2", bufs=1) as pw2:
            wq_sb = pw2.tile([P, NDC, D], f16, tag="wq")
            nc.sync.dma_start(out=wq_sb, in_=w_q[:])
            wo_sb = pw2.tile([P, NJC, D], f16, tag="wo")
            nc.sync.dma_start(out=wo_sb, in_=w_o[:])

            xq_b = {}
            for b in range(NB):
                # whole-batch transposed x, one contiguous DMA; scale by s_x
                xq = pw2.tile([P, NDC, T], f16, tag="xq", bufs=2)
                nc.sync.dma_start(out=xq, in_=xT[b])
                for c in range(NDC):
                    nc.vector.tensor_scalar_mul(xq[:, c, :], xq[:, c, :],
                                                s_x[:, c, b:b + 1])
                xq_b[b] = xq

            NCHUNK = NB * NCH
            st_q = {}    # chunk -> (q tile, qst psum)
            st_o = {}    # chunk -> o tile
            st_den = {}  # chunk -> denf tile

            def qproj_block(ck, jc):
                b = ck // NCH
                tsl = slice((ck % NCH) * CH, (ck % NCH + 1) * CH)
                if jc == 0:
                    qtile = pw2.tile([P, NJC, CH], f16, tag="q", bufs=2,
                                     name="qtile")
                    qstat = ps.tile([NH, CH], f32, tag="stat", bufs=1,
                                    name="stat")
                    st_q[ck] = (qtile, qstat)
                q, qst = st_q[ck]
                qps = psmm()
                for c in range(NDC):
                    mm(qps, wq_sb[:, c, jc * P:(jc + 1) * P],
                       xq_b[b][:, c, tsl], start=(c == 0), stop=(c == NDC - 1))
                nc.scalar.copy(out=q[:, jc, :], in_=qps)
                qsq = small.tile([P, CH], f16, tag="qsq")
                nc.vector.tensor_mul(qsq[:], q[:, jc, :], q[:, jc, :])
                mm(qst, indT_sb[:, jc, :], qsq,
                   start=(jc == 0), stop=(jc == NJC - 1))

            def qnorm_tail(ck):
                # q *= qsc/sqrt(|q|^2+eps) per head (lnqsc includes 1/8)
                q, qst = st_q[ck]
                gql = small.tile([NH, CH], f32, tag="gql")
                nc.scalar.activation(out=gql, in_=qst, func=Ln,
                                     bias=eps_t[:NH], scale=1.0)
                gqT = small.tile([NH, CH], f16, tag="gqT")
                nc.scalar.activation(out=gqT, in_=gql, func=Exp,
                                     bias=lnqsc_sb, scale=-0.5)
                for jc in range(NJC):
                    gqb = psmm()
                    mm(gqb, ind_sb[:, jc, :], gqT, start=True, stop=True)
                    nc.vector.tensor_tensor(q[:, jc, :], q[:, jc, :], gqb, MULT)

            def attn_head(ck, h):
                b = ck // NCH
                kt, vt = kT_sb[b], v_sb[b]
                q, _ = st_q[ck]
                if h == 0:
                    otile = pw2.tile([P, NJC, CH], f16, tag="o", bufs=2,
                                     name="otile")
                    dtile = small.tile([NH, CH], f32, tag="denf", name="denf")
                    st_o[ck] = otile
                    st_den[ck] = dtile
                o, denf = st_o[ck], st_den[ck]
                jc, hf = h // 2, h % 2
                r0, r1 = hf * D_HEAD, (hf + 1) * D_HEAD
                sc = ps.tile([P, NLC, CH], f32, tag="sc", bufs=2, name="sc")
                for lc in range(NLC):
                    mm(sc[:, lc, :], kt[r0:r1, jc, lc * P:(lc + 1) * P],
                       q[r0:r1, jc, :], start=True, stop=True)
                E = small.tile([P, NLC, CH], f16, tag="E")
                nc.scalar.activation(out=E[:], in_=sc[:], func=Exp, scale=1.0)
                oap = psmm()
                for lc in range(NLC):
                    mm(oap[:D_HEAD + 1, :], vt[:, lc, h, :], E[:, lc, :],
                       start=(lc == 0), stop=(lc == NLC - 1))
                dtmp = small.tile([1, CH], f32, tag="dtmp", bufs=4, name="dtmp")
                if h % 2 == 0:
                    nc.scalar.copy(out=o[r0:r1, jc, :], in_=oap[:D_HEAD, :])
                    nc.vector.tensor_copy(out=dtmp,
                                          in_=oap[D_HEAD:D_HEAD + 1, :])
                else:
                    nc.vector.tensor_copy(out=o[r0:r1, jc, :],
                                          in_=oap[:D_HEAD, :])
                    nc.scalar.copy(out=dtmp, in_=oap[D_HEAD:D_HEAD + 1, :])
                nc.sync.dma_start(out=denf[h:h + 1, :], in_=dtmp)

            def den_divide(ck):
                o, denf = st_o[ck], st_den[ck]
                denr32 = small.tile([NH, CH], f32, tag="denr32")
                nc.vector.reciprocal_approx_fast(out=denr32, in_=denf)
                denr = small.tile([NH, CH], f16, tag="denr")
                nc.vector.tensor_copy(out=denr, in_=denr32)
                for jc in range(NJC):
                    dbp = psmm()
                    mm(dbp, ind_sb[:, jc, :], denr, start=True, stop=True)
                    nc.vector.tensor_tensor(o[:, jc, :], o[:, jc, :], dbp, MULT)

            def out_block(ck, t4):
                b, th = ck // NCH, ck % NCH
                o = st_o[ck]
                trow = th * CH + t4 * P
                xs = small.tile([P, D], f32, tag="xs")
                nc.sync.dma_start(out=xs, in_=x[b, trow:trow + P, :])
                os_ = small.tile([P, D], f32, tag="os")
                for d2 in range(2):
                    ops = psmm()
                    for jc in range(NJC):
                        mm(ops, o[:, jc, t4 * P:(t4 + 1) * P],
                           wo_sb[:, jc, d2 * CH:(d2 + 1) * CH],
                           start=(jc == 0), stop=(jc == NJC - 1))
                    nc.vector.tensor_tensor(os_[:, d2 * CH:(d2 + 1) * CH],
                                            ops, xs[:, d2 * CH:(d2 + 1) * CH],
                                            ADD)
                nc.sync.dma_start(out=out[b, trow:trow + P, :], in_=os_)

            # prologue: chunk 0's projection stands alone
            for jc in range(NJC):
                qproj_block(0, jc)
            qnorm_tail(0)
            # steady state: window c = attention(c) + qproj(c+1) + out(c-1)
            for ck in range(NCHUNK):
                for h in range(NH):
                    attn_head(ck, h)
                    if ck + 1 < NCHUNK and h % 2 == 1:
                        qproj_block(ck + 1, (h - 1) // 2)
                    if ck >= 1 and h % 4 == 3:
                        out_block(ck - 1, (h - 3) // 4)
                den_divide(ck)
                if ck + 1 < NCHUNK:
                    qnorm_tail(ck + 1)
            for t4 in range(CH // P):
                out_block(NCHUNK - 1, t4)

    nc.compile()
    _bacc_mod.get_activation_tables = _orig_gat
    return nc


def _swizzle_w(wT, ncols):
    # [K, J] -> [128, K//128, J] partition-major
    K, J = wT.shape
    return np.ascontiguousarray(
        wT.reshape(K // 128, 128, J).transpose(1, 0, 2)).astype(np.float16)


def _prep_inputs(x, cond, crossattn_cond, crossattn_mask, w_norm, w_q, w_cnorm,
                 w_kv, qk_scale, w_o):
    """Shard + lay out the full inputs into 8 per-core input maps."""
    f = np.float32
    h = np.float16
    P = 128
    NDC = D // P
    sqc = np.sqrt(qk_scale.astype(f))
    shared = {
        "w_n": _swizzle_w(np.ascontiguousarray(w_norm.T), CF),
        "w_c": _swizzle_w(np.ascontiguousarray(w_cnorm.T), CF),
        "w_q": _swizzle_w(np.ascontiguousarray(w_q.T), D),
        "w_kv": _swizzle_w(np.ascontiguousarray(w_kv.T), DC),
        "w_o": _swizzle_w(np.ascontiguousarray(w_o.T), D),
        "ind": np.kron(np.eye(NH, dtype=h),
                       np.ones((1, D_HEAD), dtype=h)).reshape(NH, NDC, P),
        "indT": np.ascontiguousarray(
            np.kron(np.eye(NH, dtype=h), np.ones((D_HEAD, 1), dtype=h))
            .reshape(NDC, P, NH).transpose(1, 0, 2)),
        "lnqsc": np.log(sqc / np.sqrt(f(D_HEAD))).reshape(NH, 1).astype(f),
        "lnksc": np.log(sqc).reshape(NH, 1).astype(f),
        "onesd": np.ones((P, 1), dtype=h),
        "onesf": np.ones((1, 1), dtype=f),
    }
    in_maps = []
    for c in range(NCORES):
        s = slice(c * NB, (c + 1) * NB)
        xc = np.ascontiguousarray(x[s], dtype=f).reshape(NB, T, D)
        xt = xc.transpose(0, 2, 1).reshape(NB, NDC, P, T).transpose(0, 2, 1, 3)
        ccc = np.ascontiguousarray(crossattn_cond[s], dtype=f)
        cct = ccc.transpose(0, 2, 1).reshape(NB, NDC, P, L).transpose(0, 2, 1, 3)
        m = {
            "x": xc,
            "xT": np.ascontiguousarray(xt).astype(h),
            "ccT": np.ascontiguousarray(cct).astype(h),
            "condP": np.ascontiguousarray(
                cond[s].T.reshape(CF // P, P, NB).transpose(1, 0, 2)).astype(h),
            "gateP": np.ascontiguousarray(
                crossattn_mask[s].astype(f).T.reshape(L // P, P, NB)
                .transpose(1, 0, 2)),
        }
        m.update(shared)
        in_maps.append(m)
    return in_maps


def _run(inputs, trace=False):
    from concourse.bass_utils import run_bass_kernel_spmd

    if "nc" not in _cached:
        _cached["nc"] = _build_nc()
    nc = _cached["nc"]
    in_maps = _prep_inputs(**inputs)
    res = run_bass_kernel_spmd(nc, in_maps, core_ids=list(range(NCORES)),
                               trace=trace)
    outs = np.concatenate([r["out"] for r in res.results], axis=0)
    return outs.reshape(N, H, W, D), res


def kernel(**inputs):
    out, _ = _run(inputs, trace=False)
    return out
